# revision 4
# baseline (speedup 1.0000x reference)
"""DescendantMax kernel for Trainium2 (8 NeuronCores, pure data parallel).

Tree structure (hardcoded from the problem spec): balanced 8-ary tree,
DEPTH=6 parent->child levels, BFS node numbering.  Level k starts at
s_k = (8^k - 1) / 7 and has 8^k nodes.  Children of the j-th node of
level k are the 8 consecutive nodes s_{k+1} + 8j ... 8j+8.  So each
level's "gather" is a contiguous reshape, and the whole computation per
batch row is a chain of 8:1 contiguous-group max reductions, each
followed by an elementwise max with the parent level's own input values.

Sharding: x is (64, 299593) f32; batch is sharded across 8 cores
(8 rows per core).  Per core:

  - The leaf level (262144 elems/row) is streamed per-row through SBUF
    in partition-major [128, 2048] layout (8 KB DMA packets): load,
    pass-through store (leaves are unmodified in the output), and a
    grouped 8:1 reduce to the level-5 maxima m5 [128, 256].
  - m5 is repacked (SBUF->SBUF DMA; both sides enumerate in linear
    level-5 order) into a row-linear [64, 4096] tile: partition
    q = 8*r + c holds elements [4096c, 4096c+4096) of row r's level 5.
  - All remaining levels (5..0) are then computed for all 8 rows at
    once in this [64, n] row-chunk layout, where every DRAM transfer is
    a single DMA with 64 large contiguous packets (16 KB at level 5)
    instead of thousands of 16-128 B packets.
  - Level 1 -> 0 needs a cross-partition group of 8, so the [64, 1]
    level-1 result is repacked into [8, 8] (one row per partition)
    first.

Loads trigger on nc.sync's HW DGE queue, stores + repacks on
nc.scalar's, so descriptor generation is spread over both queues and
the 16 DMA engines stay fed.
"""

import numpy as np

BRANCH = 8
DEPTH = 6
BATCH = 64
N_CORES = 8
ROWS = BATCH // N_CORES  # rows per core
# starts[k] = (8^k - 1) // 7 ; starts[DEPTH+1] == total node count
STARTS = [(BRANCH**k - 1) // (BRANCH - 1) for k in range(DEPTH + 2)]
N_NODES = STARTS[DEPTH + 1]  # 299593

_cache: dict = {}


def _build_nc():
    import concourse.bacc as bacc
    import concourse.mybir as mybir
    from concourse.tile import TileContext

    f32 = mybir.dt.float32
    AX = mybir.AxisListType.X

    # Bacc (not raw Bass): its compile() pipeline runs
    # generate_event_semaphores, which splits multi-wait sync_info into
    # EventSemaphore insts — TRN2 allows at most 1 wait per instruction.
    nc = bacc.Bacc(None, target_bir_lowering=False)
    x = nc.dram_tensor("x", [ROWS, N_NODES], f32, kind="ExternalInput")
    out = nc.dram_tensor("out", [ROWS, N_NODES], f32, kind="ExternalOutput")

    def rowchunk(t, lvl, chunks=8):
        """DRAM AP for all rows of a level, enumerated to pair with a
        [ROWS*chunks, n/chunks] SBUF tile (row-chunk layout)."""
        a, b = STARTS[lvl], STARTS[lvl + 1]
        n = b - a
        if n // chunks == 0:
            return t[:, a:b]
        return t[:, a:b].rearrange("r (c f) -> r c f", c=chunks)

    with TileContext(nc) as tc:
        with (
            tc.tile_pool(name="big", bufs=4) as big,
            tc.tile_pool(name="mid", bufs=3) as mid,
            tc.tile_pool(name="tail", bufs=1) as tailp,
        ):
            # row-linear level-5 values for all rows: partition 8r+c
            # holds elements [4096c, 4096(c+1)) of row r's level 5
            tail5 = tailp.tile([64, 4096], f32)
            for r in range(ROWS):
                # leaves: load, pass through to out, 8:1 reduce to m5
                t6 = big.tile([128, 2048], f32, tag="t6")
                nc.sync.dma_start(
                    out=t6[:, :],
                    in_=x[r, STARTS[6] : STARTS[7]].rearrange(
                        "(p f) -> p f", p=128
                    ),
                )
                nc.scalar.dma_start(
                    out=out[r, STARTS[6] : STARTS[7]].rearrange(
                        "(p f) -> p f", p=128
                    ),
                    in_=t6[:, :],
                )
                m5 = mid.tile([128, 256], f32, tag="m5")
                nc.vector.reduce_max(
                    out=m5[:, :],
                    in_=t6[:, :].rearrange("p (g e) -> p g e", e=8),
                    axis=AX,
                )
                # repack partition-major m5 -> row-linear slot of tail5
                # (both APs enumerate in linear level-5 element order)
                nc.scalar.dma_start(
                    out=tail5[8 * r : 8 * r + 8, :], in_=m5[:, :]
                )

            # levels 5 -> 0 for all rows at once in [64, n] layout
            prev = tail5  # level-5 child maxima, row-chunk layout
            for lvl in (5, 4, 3, 2, 1):
                n = BRANCH**lvl // 8  # elems per partition at this level
                xl = tailp.tile([64, n], f32, tag=f"x{lvl}t")
                nc.sync.dma_start(out=xl[:, :], in_=rowchunk(x, lvl))
                o = tailp.tile([64, n], f32, tag=f"o{lvl}t")
                nc.vector.tensor_max(out=o[:, :], in0=prev[:, :], in1=xl[:, :])
                nc.scalar.dma_start(out=rowchunk(out, lvl), in_=o[:, :])
                if lvl > 1:
                    m = tailp.tile([64, n // 8], f32, tag=f"m{lvl - 1}t")
                    nc.vector.reduce_max(
                        out=m[:, :],
                        in_=o[:, :].rearrange("q (g e) -> q g e", e=8),
                        axis=AX,
                    )
                    prev = m
                else:
                    # level 1 -> 0: groups of 8 span partitions; repack
                    # [64, 1] -> [8, 8] (one row per partition)
                    t1 = tailp.tile([ROWS, 8], f32)
                    nc.scalar.dma_start(out=t1[:, :], in_=o[:, :])
                    m0 = tailp.tile([ROWS, 1], f32)
                    nc.vector.reduce_max(
                        out=m0[:, :],
                        in_=t1[:, :].rearrange("q (g e) -> q g e", e=8),
                        axis=AX,
                    )
                    x0 = tailp.tile([ROWS, 1], f32)
                    nc.sync.dma_start(out=x0[:, :], in_=x[:, 0:1])
                    o0 = tailp.tile([ROWS, 1], f32)
                    nc.vector.tensor_max(
                        out=o0[:, :], in0=m0[:, :], in1=x0[:, :]
                    )
                    nc.scalar.dma_start(out=out[:, 0:1], in_=o0[:, :])
    nc.compile()
    return nc


def _get_nc():
    if "nc" not in _cache:
        _cache["nc"] = _build_nc()
    return _cache["nc"]


def kernel(x, level_parents=None, level_children=None, **_ignored):
    from concourse.bass_utils import run_bass_kernel_spmd

    x = np.ascontiguousarray(np.asarray(x), dtype=np.float32)
    assert x.shape == (BATCH, N_NODES), x.shape

    nc = _get_nc()
    core_ids = list(range(N_CORES))
    in_maps = [
        {"x": x[i * ROWS : (i + 1) * ROWS]} for i in range(N_CORES)
    ]
    res = run_bass_kernel_spmd(nc, in_maps, core_ids)
    return np.concatenate([res.results[i]["out"] for i in range(N_CORES)], axis=0)
